# revision 1
# baseline (speedup 1.0000x reference)
"""Mutual channel attention (sparse_attention) TRN2 Bass kernel.

Problem: x1, x2 of shape (16, 512, 64, 64) fp32.
  q = x1.reshape(B, C, D), k = x2.reshape(B, C, D), D = 4096, scale = 1/64
  S   = q @ k^T * scale                       [B, 512, 512]
  outA = softmax_rows(S) @ k                  -> (16, 512, 64, 64)
  outB = softmax_rows(S^T) @ q                -> (16, 512, 64, 64)

Key algebra: without max-subtraction (scores ~ N(0,1), safe in fp32),
P = exp(S*scale) serves BOTH directions; only the normalization sums
differ (row sums of P for A, column sums of P for B).
  outA[c,:] = (P @ k)[c,:]   / rowsum_P[c]
  outB[e,:] = (P^T @ q)[e,:] / colsum_P[e]

Sharding: pure data parallel, 2 batches per core across 8 cores.

All matmuls run in float32r (single-pass fp32, 1 cycle/row at N=512,
~2e-4 rel err vs the fp32 reference on HW). q/k live in SBUF as 4x4
quarter tiles [128, 1024] so slots free progressively during the
d-outer out phase and the next batch's loads overlap compute.

Per-core per-batch schedule:
  1. Load q,k quarter tiles (quarter-major: the scores phase can start
     after the first 4.2MB lands).
  2. Scores: per 128-wide d-chunk, PE-transpose 4 q-blocks + 4
     k-blocks into [128,512] psum staging, copy to SBUF (q-half on
     DVE, k-half on ACT), 4 accumulating matmuls into resident S banks.
  3. exp via ScalarE with fused *1/64 scale and fused row-sum.
  4. PE-transpose P -> P_ec with fused column-sum on the copy-out.
  5. out_a = P_ec.T @ k (d-outer, frees k quarters early for the next
     batch's k loads), then out_b = P_ce.T @ q (same for q);
     normalization folded into the PSUM->SBUF copy as a per-partition
     scale; copies alternate DVE/ACT.
"""

import numpy as np

B, C, D = 16, 512, 4096
N_CORES = 8
B_PER_CORE = B // N_CORES  # 2
CC = C // 128  # 4 c-chunks
DC = D // 128  # 32 d-chunks
NQ = 8  # d-slices per row-chunk tile ([128,512] eighths: halves first-load wait, finer frees)
QW = D // NQ  # 1024 quarter width
NG = D // 512  # 8 d-groups of 512 in the out phase

_COMPILED = {}


def _build():
    import concourse.mybir as mybir
    from concourse import bacc, tile

    f32 = mybir.dt.float32
    f32r = mybir.dt.float32r
    bf16 = mybir.dt.bfloat16
    AF = mybir.ActivationFunctionType
    ROWS = B_PER_CORE * C  # 1024

    nc = bacc.Bacc(None, target_bir_lowering=False)
    x1 = nc.declare_dram_parameter("x1", [ROWS, D], f32r, isOutput=False)
    x2 = nc.declare_dram_parameter("x2", [ROWS, D], f32r, isOutput=False)
    ident = nc.declare_dram_parameter("ident", [128, 128], f32r, isOutput=False)
    outA = nc.declare_dram_parameter("outA", [ROWS, D], f32, isOutput=True)
    outB = nc.declare_dram_parameter("outB", [ROWS, D], f32, isOutput=True)

    with tile.TileContext(nc) as tc:
        with (
            tc.tile_pool(name="const", bufs=1) as constp,
            tc.tile_pool(name="qk", bufs=1) as qk,
            tc.tile_pool(name="stg_sb", bufs=3) as stgsb,
            tc.tile_pool(name="pp", bufs=2) as pp,
            tc.tile_pool(name="rp", bufs=2) as rp,
            tc.tile_pool(name="osb", bufs=6) as osb,
            tc.tile_pool(name="sps", bufs=1, space="PSUM") as sps,
            tc.tile_pool(name="stgps", bufs=4, space="PSUM") as stgps,
        ):
            idt = constp.tile([128, 128], f32r)
            nc.sync.dma_start(idt[:], ident[:])

            for b in range(B_PER_CORE):
                r0 = b * C
                # ---- load q, k as quarter tiles, quarter-major ----
                q = [[None] * NQ for _ in range(CC)]
                k = [[None] * NQ for _ in range(CC)]
                for h in range(NQ):
                    for cc in range(CC):
                        rows = slice(r0 + cc * 128, r0 + (cc + 1) * 128)
                        cols = slice(h * QW, (h + 1) * QW)
                        qt = qk.tile(
                            [128, QW], f32r, tag=f"q{cc}_{h}", name=f"q{cc}_{h}"
                        )
                        kt = qk.tile(
                            [128, QW], f32r, tag=f"k{cc}_{h}", name=f"k{cc}_{h}"
                        )
                        nc.sync.dma_start(qt[:], x1[rows, cols])
                        nc.sync.dma_start(kt[:], x2[rows, cols])
                        q[cc][h] = qt
                        k[cc][h] = kt

                # ---- scores: S_ce[cc] accumulates over 32 d-chunks ----
                s_ps = [
                    sps.tile([128, C], f32, tag=f"s{cc}", name=f"s{cc}")
                    for cc in range(CC)
                ]
                for dc in range(DC):
                    h, off = divmod(dc * 128, QW)
                    dsl = slice(off, off + 128)
                    qt_ps = stgps.tile([128, 512], f32r, tag="st", name="qt_ps")
                    kt_ps = stgps.tile([128, 512], f32r, tag="st", name="kt_ps")
                    for cc in range(CC):
                        csl = slice(cc * 128, (cc + 1) * 128)
                        nc.tensor.transpose(qt_ps[:, csl], q[cc][h][:, dsl], idt[:])
                        nc.tensor.transpose(kt_ps[:, csl], k[cc][h][:, dsl], idt[:])
                    qt_sb = stgsb.tile([128, 512], f32r, tag="qt_sb", name="qt_sb")
                    kt_sb = stgsb.tile([128, 512], f32r, tag="kt_sb", name="kt_sb")
                    nc.vector.tensor_copy(qt_sb[:], qt_ps[:])
                    nc.scalar.activation(kt_sb[:], kt_ps[:], AF.Copy)
                    for cc in range(CC):
                        nc.tensor.matmul(
                            s_ps[cc][:],
                            qt_sb[:, cc * 128 : (cc + 1) * 128],
                            kt_sb[:],
                            start=(dc == 0),
                            stop=(dc == DC - 1),
                        )

                # ---- exp + row sums (direction A) ----
                p_ce = []
                rinv_a = []
                for cc in range(CC):
                    p = pp.tile([128, C], f32r, tag=f"pce{cc}", name=f"pce{cc}")
                    rs = rp.tile([128, 1], f32, tag=f"rsa{cc}", name=f"rsa{cc}")
                    nc.scalar.activation(
                        p[:], s_ps[cc][:], AF.Exp, scale=1.0 / 64.0, accum_out=rs[:]
                    )
                    ri = rp.tile([128, 1], f32, tag=f"ria{cc}", name=f"ria{cc}")
                    nc.vector.reciprocal(ri[:], rs[:])
                    p_ce.append(p)
                    rinv_a.append(ri)

                # ---- transpose P -> P_ec + column sums (direction B) ----
                p_ec = []
                rinv_b = []
                for ec in range(CC):
                    esl = slice(ec * 128, (ec + 1) * 128)
                    t_ps = stgps.tile([128, 512], f32r, tag="st", name="pt_ps")
                    for cc in range(CC):
                        nc.tensor.transpose(
                            t_ps[:, cc * 128 : (cc + 1) * 128], p_ce[cc][:, esl], idt[:]
                        )
                    p = pp.tile([128, C], f32r, tag=f"pec{ec}", name=f"pec{ec}")
                    rs = rp.tile([128, 1], f32, tag=f"rsb{ec}", name=f"rsb{ec}")
                    nc.scalar.activation(p[:], t_ps[:], AF.Copy, accum_out=rs[:])
                    ri = rp.tile([128, 1], f32, tag=f"rib{ec}", name=f"rib{ec}")
                    nc.vector.reciprocal(ri[:], rs[:])
                    p_ec.append(p)
                    rinv_b.append(ri)

                # ---- out_a = (P_ec.T @ k) * rinv_a, d-outer frees k early ----
                for g in range(NG):
                    h, off = divmod(g * 512, QW)
                    dsl = slice(off, off + 512)
                    for cc in range(CC):
                        csl = slice(cc * 128, (cc + 1) * 128)
                        o_ps = stgps.tile([128, 512], f32, tag="st", name="oa_ps")
                        for ec in range(CC):
                            nc.tensor.matmul(
                                o_ps[:],
                                p_ec[ec][:, csl],
                                k[ec][h][:, dsl],
                                start=(ec == 0),
                                stop=(ec == CC - 1),
                            )
                        o_sb = osb.tile([128, 512], f32, tag="osb", name="oa_sb")
                        if cc % 2 == 0:
                            nc.vector.tensor_scalar_mul(o_sb[:], o_ps[:], rinv_a[cc][:])
                        else:
                            nc.scalar.activation(
                                o_sb[:], o_ps[:], AF.Copy, scale=rinv_a[cc][:]
                            )
                        nc.sync.dma_start(
                            outA[
                                r0 + cc * 128 : r0 + (cc + 1) * 128,
                                g * 512 : (g + 1) * 512,
                            ],
                            o_sb[:],
                        )

                # ---- out_b = (P_ce.T @ q) * rinv_b, d-outer frees q early ----
                for g in range(NG):
                    h, off = divmod(g * 512, QW)
                    dsl = slice(off, off + 512)
                    for ec in range(CC):
                        esl = slice(ec * 128, (ec + 1) * 128)
                        o_ps = stgps.tile([128, 512], f32, tag="st", name="ob_ps")
                        for cc in range(CC):
                            nc.tensor.matmul(
                                o_ps[:],
                                p_ce[cc][:, esl],
                                q[cc][h][:, dsl],
                                start=(cc == 0),
                                stop=(cc == CC - 1),
                            )
                        o_sb = osb.tile([128, 512], f32, tag="osb", name="ob_sb")
                        if ec % 2 == 0:
                            nc.vector.tensor_scalar_mul(o_sb[:], o_ps[:], rinv_b[ec][:])
                        else:
                            nc.scalar.activation(
                                o_sb[:], o_ps[:], AF.Copy, scale=rinv_b[ec][:]
                            )
                        nc.sync.dma_start(
                            outB[
                                r0 + ec * 128 : r0 + (ec + 1) * 128,
                                g * 512 : (g + 1) * 512,
                            ],
                            o_sb[:],
                        )

    nc.finalize()
    return nc


def _get_nc():
    if "nc" not in _COMPILED:
        _COMPILED["nc"] = _build()
    return _COMPILED["nc"]


def kernel(x1: np.ndarray, x2: np.ndarray):
    from concourse.bass_utils import run_bass_kernel_spmd

    nc = _get_nc()
    x1 = np.ascontiguousarray(x1, dtype=np.float32)
    x2 = np.ascontiguousarray(x2, dtype=np.float32)
    ident = np.eye(128, dtype=np.float32)

    in_maps = []
    for i in range(N_CORES):
        sl = slice(i * B_PER_CORE, (i + 1) * B_PER_CORE)
        in_maps.append(
            {
                "x1": x1[sl].reshape(B_PER_CORE * C, D),
                "x2": x2[sl].reshape(B_PER_CORE * C, D),
                "ident": ident,
            }
        )

    res = None
    for attempt in range(3):
        try:
            res = run_bass_kernel_spmd(nc, in_maps, list(range(N_CORES))).results
            break
        except Exception:
            if attempt == 2:
                raise
    assert res is not None

    outA = np.empty((B, C, 64, 64), dtype=np.float32)
    outB = np.empty((B, C, 64, 64), dtype=np.float32)
    for i in range(N_CORES):
        sl = slice(i * B_PER_CORE, (i + 1) * B_PER_CORE)
        outA[sl] = res[i]["outA"].reshape(B_PER_CORE, C, 64, 64)
        outB[sl] = res[i]["outB"].reshape(B_PER_CORE, C, 64, 64)
    return outA, outB



# revision 2
# speedup vs baseline: 1.3508x; 1.3508x over previous
"""Mutual channel attention (sparse_attention) TRN2 Bass kernel.

Problem: x1, x2 of shape (16, 512, 64, 64) fp32.
  q = x1.reshape(B, C, D), k = x2.reshape(B, C, D), D = 4096, scale = 1/64
  S   = q @ k^T * scale                       [B, 512, 512]
  outA = softmax_rows(S) @ k                  -> (16, 512, 64, 64)
  outB = softmax_rows(S^T) @ q                -> (16, 512, 64, 64)

Key algebra: without max-subtraction (scores ~ N(0,1), safe in fp32),
P = exp(S*scale) serves BOTH directions; only the normalization sums
differ (row sums of P for A, column sums of P for B).

Sharding: pure data parallel, 2 batches per core across 8 cores.

v2 design (vs the all-on-PE fp32r baseline at ~285us):
  * The PE array was the bottleneck at 86% occupancy, with 544 PE
    transposes per core (~47us) on top of the ~164us of irreducible
    matmul streaming. All operand layout prep is moved to the host:
    inputs are shipped bf16 in BOTH layouts (natural [c,d] for the
    out-phase moving operands, d-major [d,c] for the scores phase), so
    the PE does only real matmuls plus a tiny 16-transpose P_ce->P_ec
    pass per batch.
  * bf16 operands everywhere on chip (fp32 PSUM accumulation).
    Precision budget: rel err ~5e-3 vs the 2e-2 gate.
  * Outputs are stored bf16 and widened to fp32 on the host, cutting
    store traffic in half: total HBM traffic 48MB/core (~140us) vs a
    ~165us PE schedule, keeping the kernel PE-bound with DMA slack.

Per-core per-batch schedule (PE program order):
  1. scores: for each 128-wide d-chunk, 4 accumulating matmuls
     (stationary qT block, moving kT row) into 4 resident PSUM banks.
  2. exp via ScalarE with fused *1/64 scale and fused row-sum (dir A).
  3. PE-transpose P_ce -> P_ec with column sums on the copy-out (dir B).
  4. outB = (P_ce.T @ q) * rinv_b, then outA = (P_ec.T @ k) * rinv_a;
     normalization folded into the PSUM->SBUF copy; copies alternate
     DVE/ACT; stores are 1MB bf16 row-blocks.
"""

import numpy as np

B, C, D = 16, 512, 4096
N_CORES = 8
B_PER_CORE = B // N_CORES  # 2
CC = C // 128  # 4 c-blocks
DC = D // 128  # 32 d-chunks
NG = 4  # qT/kT load groups per batch
GD = DC // NG  # 8 d-chunks per load group
ND = D // 512  # 8 moving 512-wide d-slices in the out phase
ROWS = B_PER_CORE * C  # 1024

_COMPILED = {}


def _build():
    import concourse.mybir as mybir
    from concourse import bacc, tile

    f32 = mybir.dt.float32
    bf16 = mybir.dt.bfloat16
    AF = mybir.ActivationFunctionType

    nc = bacc.Bacc(None, target_bir_lowering=False)
    # Natural layouts [b*C + c, d] (moving operands for the out phase).
    qn = nc.declare_dram_parameter("qn", [ROWS, D], bf16, isOutput=False)
    kn = nc.declare_dram_parameter("kn", [ROWS, D], bf16, isOutput=False)
    # d-major layouts [d % 128, b*DC + d//128, c] (scores operands).
    qT = nc.declare_dram_parameter("qT", [128, B_PER_CORE * DC, C], bf16, isOutput=False)
    kT = nc.declare_dram_parameter("kT", [128, B_PER_CORE * DC, C], bf16, isOutput=False)
    ident = nc.declare_dram_parameter("ident", [128, 128], bf16, isOutput=False)
    outA = nc.declare_dram_parameter("outA", [ROWS, D], bf16, isOutput=True)
    outB = nc.declare_dram_parameter("outB", [ROWS, D], bf16, isOutput=True)

    with tile.TileContext(nc) as tc:
        with (
            tc.tile_pool(name="const", bufs=1) as constp,
            tc.tile_pool(name="qkT", bufs=1) as qkt,
            tc.tile_pool(name="qk", bufs=1) as qk,
            tc.tile_pool(name="pp", bufs=2) as pp,
            tc.tile_pool(name="rp", bufs=2) as rp,
            tc.tile_pool(name="osb", bufs=3) as osb,
            tc.tile_pool(name="sps", bufs=1, space="PSUM") as sps,
            tc.tile_pool(name="ptps", bufs=2, space="PSUM") as ptps,
            tc.tile_pool(name="ops", bufs=2, space="PSUM") as ops,
        ):
            idt = constp.tile([128, 128], bf16)
            nc.sync.dma_start(idt[:], ident[:])

            for b in range(B_PER_CORE):
                r0 = b * C
                # ---- loads: scores operands first (g-major), then the
                # out-phase moving operands ----
                qt_t = []
                kt_t = []
                for g in range(NG):
                    qt = qkt.tile([128, GD * C], bf16, tag=f"qT{g}", name=f"qT{g}")
                    kt = qkt.tile([128, GD * C], bf16, tag=f"kT{g}", name=f"kT{g}")
                    csl = slice(b * DC + g * GD, b * DC + (g + 1) * GD)
                    nc.sync.dma_start(qt[:], qT[:, csl, :])
                    nc.sync.dma_start(kt[:], kT[:, csl, :])
                    qt_t.append(qt)
                    kt_t.append(kt)
                q_t = []
                k_t = []
                for cc in range(CC):
                    qt = qk.tile([128, D], bf16, tag=f"q{cc}", name=f"q{cc}")
                    nc.sync.dma_start(qt[:], qn[r0 + cc * 128 : r0 + (cc + 1) * 128, :])
                    q_t.append(qt)
                for cc in range(CC):
                    kt = qk.tile([128, D], bf16, tag=f"k{cc}", name=f"k{cc}")
                    nc.sync.dma_start(kt[:], kn[r0 + cc * 128 : r0 + (cc + 1) * 128, :])
                    k_t.append(kt)

                # ---- scores: S_ce[cc] accumulates over 32 d-chunks ----
                s_ps = [
                    sps.tile([128, C], f32, tag=f"s{cc}", name=f"s{cc}")
                    for cc in range(CC)
                ]
                for dc in range(DC):
                    g, j = divmod(dc, GD)
                    rhs = kt_t[g][:, j * C : (j + 1) * C]
                    for cc in range(CC):
                        nc.tensor.matmul(
                            s_ps[cc][:],
                            qt_t[g][:, j * C + cc * 128 : j * C + (cc + 1) * 128],
                            rhs,
                            start=(dc == 0),
                            stop=(dc == DC - 1),
                        )

                # ---- exp + row sums (direction A) ----
                p_ce = []
                rinv_a = []
                for cc in range(CC):
                    p = pp.tile([128, C], bf16, tag=f"pce{cc}", name=f"pce{cc}")
                    rs = rp.tile([128, 1], f32, tag=f"rsa{cc}", name=f"rsa{cc}")
                    nc.scalar.activation(
                        p[:], s_ps[cc][:], AF.Exp, scale=1.0 / 64.0, accum_out=rs[:]
                    )
                    ri = rp.tile([128, 1], f32, tag=f"ria{cc}", name=f"ria{cc}")
                    nc.vector.reciprocal(ri[:], rs[:])
                    p_ce.append(p)
                    rinv_a.append(ri)

                # ---- transpose P -> P_ec + column sums (direction B) ----
                p_ec = []
                rinv_b = []
                for ec in range(CC):
                    esl = slice(ec * 128, (ec + 1) * 128)
                    t_ps = ptps.tile([128, C], bf16, tag="pt", name="pt_ps")
                    for cc in range(CC):
                        nc.tensor.transpose(
                            t_ps[:, cc * 128 : (cc + 1) * 128], p_ce[cc][:, esl], idt[:]
                        )
                    p = pp.tile([128, C], bf16, tag=f"pec{ec}", name=f"pec{ec}")
                    rs = rp.tile([128, 1], f32, tag=f"rsb{ec}", name=f"rsb{ec}")
                    nc.scalar.activation(p[:], t_ps[:], AF.Copy, accum_out=rs[:])
                    ri = rp.tile([128, 1], f32, tag=f"rib{ec}", name=f"rib{ec}")
                    nc.vector.reciprocal(ri[:], rs[:])
                    p_ec.append(p)
                    rinv_b.append(ri)

                # ---- outB = (P_ce.T @ q) * rinv_b ----
                for eb in range(CC):
                    esl = slice(eb * 128, (eb + 1) * 128)
                    ot = osb.tile([128, D], bf16, tag="osb", name="ob_sb")
                    for g in range(ND):
                        dsl = slice(g * 512, (g + 1) * 512)
                        o_ps = ops.tile([128, 512], f32, tag="o", name="ob_ps")
                        for cc in range(CC):
                            nc.tensor.matmul(
                                o_ps[:],
                                p_ce[cc][:, esl],
                                q_t[cc][:, dsl],
                                start=(cc == 0),
                                stop=(cc == CC - 1),
                            )
                        if g % 2 == 0:
                            nc.vector.tensor_scalar_mul(
                                ot[:, dsl], o_ps[:], rinv_b[eb][:]
                            )
                        else:
                            nc.scalar.activation(
                                ot[:, dsl], o_ps[:], AF.Copy, scale=rinv_b[eb][:]
                            )
                    nc.sync.dma_start(outB[r0 + eb * 128 : r0 + (eb + 1) * 128, :], ot[:])

                # ---- outA = (P_ec.T @ k) * rinv_a ----
                for cb in range(CC):
                    csl = slice(cb * 128, (cb + 1) * 128)
                    ot = osb.tile([128, D], bf16, tag="osb", name="oa_sb")
                    for g in range(ND):
                        dsl = slice(g * 512, (g + 1) * 512)
                        o_ps = ops.tile([128, 512], f32, tag="o", name="oa_ps")
                        for ec in range(CC):
                            nc.tensor.matmul(
                                o_ps[:],
                                p_ec[ec][:, csl],
                                k_t[ec][:, dsl],
                                start=(ec == 0),
                                stop=(ec == CC - 1),
                            )
                        if g % 2 == 0:
                            nc.vector.tensor_scalar_mul(
                                ot[:, dsl], o_ps[:], rinv_a[cb][:]
                            )
                        else:
                            nc.scalar.activation(
                                ot[:, dsl], o_ps[:], AF.Copy, scale=rinv_a[cb][:]
                            )
                    nc.sync.dma_start(outA[r0 + cb * 128 : r0 + (cb + 1) * 128, :], ot[:])

    nc.finalize()
    return nc


def _get_nc():
    if "nc" not in _COMPILED:
        _COMPILED["nc"] = _build()
    return _COMPILED["nc"]


def make_in_maps(x1: np.ndarray, x2: np.ndarray):
    """Host-side layout prep: slice per core, cast to bf16, and build both
    the natural [c, d] and d-major [d%128, chunk, c] layouts."""
    import ml_dtypes

    bf = ml_dtypes.bfloat16
    x1 = np.asarray(x1, dtype=np.float32).reshape(B, C, D)
    x2 = np.asarray(x2, dtype=np.float32).reshape(B, C, D)
    ident = np.eye(128, dtype=bf)

    in_maps = []
    for i in range(N_CORES):
        sl = slice(i * B_PER_CORE, (i + 1) * B_PER_CORE)
        xb1 = x1[sl].astype(bf)  # [2, 512, 4096]
        xb2 = x2[sl].astype(bf)
        # d-major: [2, C, DC, 128] -> [128, 2, DC, C] -> [128, 2*DC, C]
        qT = np.ascontiguousarray(
            xb1.reshape(B_PER_CORE, C, DC, 128).transpose(3, 0, 2, 1)
        ).reshape(128, B_PER_CORE * DC, C)
        kT = np.ascontiguousarray(
            xb2.reshape(B_PER_CORE, C, DC, 128).transpose(3, 0, 2, 1)
        ).reshape(128, B_PER_CORE * DC, C)
        in_maps.append(
            {
                "qn": np.ascontiguousarray(xb1.reshape(ROWS, D)),
                "kn": np.ascontiguousarray(xb2.reshape(ROWS, D)),
                "qT": qT,
                "kT": kT,
                "ident": ident,
            }
        )
    return in_maps


def kernel(x1: np.ndarray, x2: np.ndarray):
    from concourse.bass_utils import run_bass_kernel_spmd

    nc = _get_nc()
    in_maps = make_in_maps(x1, x2)

    res = None
    for attempt in range(3):
        try:
            res = run_bass_kernel_spmd(nc, in_maps, list(range(N_CORES))).results
            break
        except Exception:
            if attempt == 2:
                raise
    assert res is not None

    outA = np.empty((B, C, 64, 64), dtype=np.float32)
    outB = np.empty((B, C, 64, 64), dtype=np.float32)
    for i in range(N_CORES):
        sl = slice(i * B_PER_CORE, (i + 1) * B_PER_CORE)
        outA[sl] = np.asarray(res[i]["outA"], dtype=np.float32).reshape(
            B_PER_CORE, C, 64, 64
        )
        outB[sl] = np.asarray(res[i]["outB"], dtype=np.float32).reshape(
            B_PER_CORE, C, 64, 64
        )
    return outA, outB


# revision 7
# speedup vs baseline: 1.3660x; 1.0113x over previous
"""Mutual channel attention (sparse_attention) TRN2 Bass kernel.

Problem: x1, x2 of shape (16, 512, 64, 64) fp32.
  q = x1.reshape(B, C, D), k = x2.reshape(B, C, D), D = 4096, scale = 1/64
  S   = q @ k^T * scale                       [B, 512, 512]
  outA = softmax_rows(S) @ k                  -> (16, 512, 64, 64)
  outB = softmax_rows(S^T) @ q                -> (16, 512, 64, 64)

Key algebra: without max-subtraction (scores ~ N(0,1), safe in fp32),
P = exp(S*scale) serves BOTH directions; only the normalization sums
differ (row sums of P for A, column sums of P for B).

Sharding: pure data parallel, 2 batches per core across 8 cores.

v2 design (vs the all-on-PE fp32r baseline at ~285us):
  * The PE array was the bottleneck at 86% occupancy, with 544 PE
    transposes per core (~47us) on top of the ~164us of irreducible
    matmul streaming. All operand layout prep is moved to the host:
    inputs are shipped bf16 in BOTH layouts (natural [c,d] for the
    out-phase moving operands, d-major [d,c] for the scores phase), so
    the PE does only real matmuls plus a tiny 16-transpose P_ce->P_ec
    pass per batch.
  * bf16 operands everywhere on chip (fp32 PSUM accumulation).
    Precision budget: rel err ~5e-3 vs the 2e-2 gate.
  * Outputs are stored bf16 and widened to fp32 on the host, cutting
    store traffic in half: total HBM traffic 48MB/core (~140us) vs a
    ~165us PE schedule, keeping the kernel PE-bound with DMA slack.

Per-core per-batch schedule (PE program order):
  1. scores: for each 128-wide d-chunk, 4 accumulating matmuls
     (stationary qT block, moving kT row) into 4 resident PSUM banks.
  2. exp via ScalarE with fused *1/64 scale and fused row-sum (dir A).
  3. PE-transpose P_ce -> P_ec with column sums on the copy-out (dir B).
  4. outB = (P_ce.T @ q) * rinv_b, then outA = (P_ec.T @ k) * rinv_a;
     normalization folded into the PSUM->SBUF copy; copies alternate
     DVE/ACT; stores are 1MB bf16 row-blocks.
"""

import numpy as np

B, C, D = 16, 512, 4096
N_CORES = 8
B_PER_CORE = B // N_CORES  # 2
CC = C // 128  # 4 c-blocks
DC = D // 128  # 32 d-chunks
NG = 8  # qT/kT load groups per batch
GD = DC // NG  # 4 d-chunks per load group
ND = D // 512  # 8 moving 512-wide d-slices in the out phase
NH = 2  # d-halves for the natural-layout q/k tiles
HW_ = D // NH  # 2048
ROWS = B_PER_CORE * C  # 1024

_COMPILED = {}


def _build():
    import concourse.mybir as mybir
    from concourse import bacc, tile

    f32 = mybir.dt.float32
    bf16 = mybir.dt.bfloat16
    AF = mybir.ActivationFunctionType

    nc = bacc.Bacc(None, target_bir_lowering=False)
    # Natural layouts [b*C + c, d] (moving operands for the out phase).
    qn = nc.declare_dram_parameter("qn", [ROWS, D], bf16, isOutput=False)
    kn = nc.declare_dram_parameter("kn", [ROWS, D], bf16, isOutput=False)
    # d-major layouts [d % 128, b*DC + d//128, c] (scores operands).
    qT = nc.declare_dram_parameter("qT", [128, B_PER_CORE * DC, C], bf16, isOutput=False)
    kT = nc.declare_dram_parameter("kT", [128, B_PER_CORE * DC, C], bf16, isOutput=False)
    ident = nc.declare_dram_parameter("ident", [128, 128], bf16, isOutput=False)
    outA = nc.declare_dram_parameter("outA", [ROWS, D], bf16, isOutput=True)
    outB = nc.declare_dram_parameter("outB", [ROWS, D], bf16, isOutput=True)

    with tile.TileContext(nc) as tc:
        with (
            tc.tile_pool(name="const", bufs=1) as constp,
            tc.tile_pool(name="qkT", bufs=1) as qkt,
            tc.tile_pool(name="qk", bufs=1) as qk,
            tc.tile_pool(name="pp", bufs=2) as pp,
            tc.tile_pool(name="rp", bufs=2) as rp,
            tc.tile_pool(name="osb", bufs=3) as osb,
            tc.tile_pool(name="sps", bufs=1, space="PSUM") as sps,
            tc.tile_pool(name="ptps", bufs=2, space="PSUM") as ptps,
            tc.tile_pool(name="ops", bufs=2, space="PSUM") as ops,
        ):
            idt = None

            for b in range(B_PER_CORE):
                r0 = b * C
                # ---- loads: scores operands first (g-major), then the
                # out-phase moving operands in d-halves (so outB can start
                # after only the first halves land) ----
                qt_t = []
                kt_t = []
                for g in range(NG):
                    qt = qkt.tile([128, GD * C], bf16, tag=f"qT{g}", name=f"qT{g}")
                    kt = qkt.tile([128, GD * C], bf16, tag=f"kT{g}", name=f"kT{g}")
                    csl = slice(b * DC + g * GD, b * DC + (g + 1) * GD)
                    nc.sync.dma_start(qt[:], qT[:, csl, :])
                    nc.sync.dma_start(kt[:], kT[:, csl, :])
                    qt_t.append(qt)
                    kt_t.append(kt)
                    if idt is None:
                        idt = constp.tile([128, 128], bf16)
                        nc.sync.dma_start(idt[:], ident[:])
                q_t = [[None] * NH for _ in range(CC)]
                k_t = [[None] * NH for _ in range(CC)]
                for h in range(NH):
                    for cc in range(CC):
                        t = qk.tile([128, HW_], bf16, tag=f"q{cc}_{h}", name=f"q{cc}_{h}")
                        nc.sync.dma_start(
                            t[:],
                            qn[
                                r0 + cc * 128 : r0 + (cc + 1) * 128,
                                h * HW_ : (h + 1) * HW_,
                            ],
                        )
                        q_t[cc][h] = t
                for h in range(NH):
                    for cc in range(CC):
                        t = qk.tile([128, HW_], bf16, tag=f"k{cc}_{h}", name=f"k{cc}_{h}")
                        nc.sync.dma_start(
                            t[:],
                            kn[
                                r0 + cc * 128 : r0 + (cc + 1) * 128,
                                h * HW_ : (h + 1) * HW_,
                            ],
                        )
                        k_t[cc][h] = t

                # ---- scores: S_ce[cc] accumulates over 32 d-chunks ----
                s_ps = [
                    sps.tile([128, C], f32, tag=f"s{cc}", name=f"s{cc}")
                    for cc in range(CC)
                ]
                for dc in range(DC):
                    g, j = divmod(dc, GD)
                    rhs = kt_t[g][:, j * C : (j + 1) * C]
                    for cc in range(CC):
                        nc.tensor.matmul(
                            s_ps[cc][:],
                            qt_t[g][:, j * C + cc * 128 : j * C + (cc + 1) * 128],
                            rhs,
                            start=(dc == 0),
                            stop=(dc == DC - 1),
                        )

                # ---- exp + row sums (direction A) ----
                p_ce = []
                rinv_a = []
                for cc in range(CC):
                    p = pp.tile([128, C], bf16, tag=f"pce{cc}", name=f"pce{cc}")
                    rs = rp.tile([128, 1], f32, tag=f"rsa{cc}", name=f"rsa{cc}")
                    nc.scalar.activation(
                        p[:], s_ps[cc][:], AF.Exp, scale=1.0 / 64.0, accum_out=rs[:]
                    )
                    ri = rp.tile([128, 1], f32, tag=f"ria{cc}", name=f"ria{cc}")
                    nc.vector.reciprocal(ri[:], rs[:])
                    p_ce.append(p)
                    rinv_a.append(ri)

                # ---- transpose P -> P_ec + column sums (direction B) ----
                p_ec = []
                rinv_b = []
                for ec in range(CC):
                    esl = slice(ec * 128, (ec + 1) * 128)
                    t_ps = ptps.tile([128, C], bf16, tag="pt", name="pt_ps")
                    for cc in range(CC):
                        nc.tensor.transpose(
                            t_ps[:, cc * 128 : (cc + 1) * 128], p_ce[cc][:, esl], idt[:]
                        )
                    p = pp.tile([128, C], bf16, tag=f"pec{ec}", name=f"pec{ec}")
                    rs = rp.tile([128, 1], f32, tag=f"rsb{ec}", name=f"rsb{ec}")
                    nc.scalar.activation(p[:], t_ps[:], AF.Copy, accum_out=rs[:])
                    ri = rp.tile([128, 1], f32, tag=f"rib{ec}", name=f"rib{ec}")
                    nc.vector.reciprocal(ri[:], rs[:])
                    p_ec.append(p)
                    rinv_b.append(ri)

                # ---- outB = (P_ce.T @ q) * rinv_b ----
                for eb in range(CC):
                    esl = slice(eb * 128, (eb + 1) * 128)
                    ot = osb.tile([128, D], bf16, tag="osb", name="ob_sb")
                    for g in range(ND):
                        h, off = divmod(g * 512, HW_)
                        dsl = slice(g * 512, (g + 1) * 512)
                        o_ps = ops.tile([128, 512], f32, tag="o", name="ob_ps")
                        for cc in range(CC):
                            nc.tensor.matmul(
                                o_ps[:],
                                p_ce[cc][:, esl],
                                q_t[cc][h][:, off : off + 512],
                                start=(cc == 0),
                                stop=(cc == CC - 1),
                            )
                        if g % 2 == 0:
                            nc.vector.tensor_scalar_mul(
                                ot[:, dsl], o_ps[:], rinv_b[eb][:]
                            )
                        else:
                            nc.scalar.activation(
                                ot[:, dsl], o_ps[:], AF.Copy, scale=rinv_b[eb][:]
                            )
                        if g % (ND // NH) == ND // NH - 1:
                            hs = slice((h * HW_), (h + 1) * HW_)
                            eng = nc.sync if h % 2 == 0 else nc.scalar
                            eng.dma_start(
                                outB[r0 + eb * 128 : r0 + (eb + 1) * 128, hs],
                                ot[:, hs],
                            )

                # ---- outA = (P_ec.T @ k) * rinv_a ----
                for cb in range(CC):
                    csl = slice(cb * 128, (cb + 1) * 128)
                    ot = osb.tile([128, D], bf16, tag="osb", name="oa_sb")
                    for g in range(ND):
                        h, off = divmod(g * 512, HW_)
                        dsl = slice(g * 512, (g + 1) * 512)
                        o_ps = ops.tile([128, 512], f32, tag="o", name="oa_ps")
                        for ec in range(CC):
                            nc.tensor.matmul(
                                o_ps[:],
                                p_ec[ec][:, csl],
                                k_t[ec][h][:, off : off + 512],
                                start=(ec == 0),
                                stop=(ec == CC - 1),
                            )
                        if g % 2 == 0:
                            nc.vector.tensor_scalar_mul(
                                ot[:, dsl], o_ps[:], rinv_a[cb][:]
                            )
                        else:
                            nc.scalar.activation(
                                ot[:, dsl], o_ps[:], AF.Copy, scale=rinv_a[cb][:]
                            )
                        if g % (ND // NH) == ND // NH - 1:
                            hs = slice((h * HW_), (h + 1) * HW_)
                            eng = nc.sync if h % 2 == 1 else nc.scalar
                            eng.dma_start(
                                outA[r0 + cb * 128 : r0 + (cb + 1) * 128, hs],
                                ot[:, hs],
                            )

    nc.finalize()
    return nc


def _get_nc():
    if "nc" not in _COMPILED:
        _COMPILED["nc"] = _build()
    return _COMPILED["nc"]


def make_in_maps(x1: np.ndarray, x2: np.ndarray):
    """Host-side layout prep: slice per core, cast to bf16, and build both
    the natural [c, d] and d-major [d%128, chunk, c] layouts."""
    import ml_dtypes

    bf = ml_dtypes.bfloat16
    x1 = np.asarray(x1, dtype=np.float32).reshape(B, C, D)
    x2 = np.asarray(x2, dtype=np.float32).reshape(B, C, D)
    ident = np.eye(128, dtype=bf)

    in_maps = []
    for i in range(N_CORES):
        sl = slice(i * B_PER_CORE, (i + 1) * B_PER_CORE)
        xb1 = x1[sl].astype(bf)  # [2, 512, 4096]
        xb2 = x2[sl].astype(bf)
        # d-major: [2, C, DC, 128] -> [128, 2, DC, C] -> [128, 2*DC, C]
        qT = np.ascontiguousarray(
            xb1.reshape(B_PER_CORE, C, DC, 128).transpose(3, 0, 2, 1)
        ).reshape(128, B_PER_CORE * DC, C)
        kT = np.ascontiguousarray(
            xb2.reshape(B_PER_CORE, C, DC, 128).transpose(3, 0, 2, 1)
        ).reshape(128, B_PER_CORE * DC, C)
        in_maps.append(
            {
                "qn": np.ascontiguousarray(xb1.reshape(ROWS, D)),
                "kn": np.ascontiguousarray(xb2.reshape(ROWS, D)),
                "qT": qT,
                "kT": kT,
                "ident": ident,
            }
        )
    return in_maps


def kernel(x1: np.ndarray, x2: np.ndarray):
    from concourse.bass_utils import run_bass_kernel_spmd

    nc = _get_nc()
    in_maps = make_in_maps(x1, x2)

    res = None
    for attempt in range(3):
        try:
            res = run_bass_kernel_spmd(nc, in_maps, list(range(N_CORES))).results
            break
        except Exception:
            if attempt == 2:
                raise
    assert res is not None

    outA = np.empty((B, C, 64, 64), dtype=np.float32)
    outB = np.empty((B, C, 64, 64), dtype=np.float32)
    for i in range(N_CORES):
        sl = slice(i * B_PER_CORE, (i + 1) * B_PER_CORE)
        outA[sl] = np.asarray(res[i]["outA"], dtype=np.float32).reshape(
            B_PER_CORE, C, 64, 64
        )
        outB[sl] = np.asarray(res[i]["outB"], dtype=np.float32).reshape(
            B_PER_CORE, C, 64, 64
        )
    return outA, outB
